# revision 2
# baseline (speedup 1.0000x reference)
"""W4A4 quantized linear (AutoQVLALinearW4A4) on 8 Trainium2 NeuronCores.

y = dequant_rowwise_quant(x) @ dequant_w4(qweight)^T + bias

2x4 sharding: 2 token groups x 4 out-feature groups. Each core gets a
[2048, 4096] slice of x (row-reversed) and a 1024-row slice of the packed
weights / scales / bias, and produces a [2048, 1024] slice of the output.
vs pure column-parallel this halves both the per-core HBM x traffic and
the SBUF-fabric transpose traffic, bringing the DMA pipeline (~85us) well
under the fp8 PE roofline (~110-125us).

Device algorithm (per core), exact-integer math on the PE:
  1. weights: ONE DMA-transpose of the packed int4 buffer straight from
     DRAM (viewed as fp16 so byte QUADS move as units): partition jj of
     quad-chunk c2 holds bytes for k = 512*c2 + 4*jj + {0,1,2,3} of all
     1024 local out-features. Nibble-unpack on ACT (high) / DVE (low)
     directly into wt_sep[jj, c', e, n], c' = 2*c2+h, e = nibble plane.
  2. amax for OWN 512 tokens (int16 abs-bit trick + max tree on DVE),
     a_scale = max/7, AllGather of the 4 group members' a_scales (2KB).
  3. qb = fp16(x * (1/a_scale) + 1536)  (exact round-half-even to int+1536)
     q8 = fp8_e4m3(qb - 1536), written with the k-swizzle sigma that makes
     the q8 byte order match the weight quad layout: byte
     512*B + 256*h + 2*jj + e  <-  k = 512*B + 4*jj + 2*h + e.
     (A pure permutation of the contraction; the matmul sum is invariant.)
  4. transpose q8 via DMA-transpose of byte-PAIRS viewed as fp16, then
     fp8 DoubleRowSwInterleave matmuls: q^T byte-pairs are the STATIONARY
     operand (SwInterleave's column reversal is cancelled by feeding x
     row-reversed from the host), wt_sep planes are the MOVING operand.
     Two PSUM banks per token tile cover the 1024 local out-features.
  5. epilogue: (psum * a_scale_pp) * wscale_bcast + bias_bcast on DVE.
     a_scale needs a partition flip (out tokens run opposite to the
     x_rev rows) -> one tiny anti-diagonal matmul (J @ s).
"""

import numpy as np
import concourse.bass as bass
import concourse.mybir as mybir
from concourse import bacc
from concourse.tile import TileContext
from concourse.bass_utils import run_bass_kernel_spmd

F8 = mybir.dt.float8e4
F16 = mybir.dt.float16
F32 = mybir.dt.float32
I8 = mybir.dt.int8
I16 = mybir.dt.int16
AOP = mybir.AluOpType
ACTF = mybir.ActivationFunctionType
SWI = mybir.MatmulPerfMode.DoubleRowSwInterleave

N_CORES = 8
TM = 2            # token groups
TN = 4            # out-feature groups


def build(ML=2048, K=4096, NS=1024, use_cc=True, mm_bufs=6, qt_bufs=8,
          x_bufs=6, n_pre=3, repeat=1, debug_dump=False, ablate=None):
    """Build + compile the per-core program. Returns the Bacc object."""
    assert ML % 128 == 0 and K % 512 == 0 and NS % 256 == 0
    T = ML // 128         # token tiles
    NH = NS // 2          # out-feature half (one PSUM bank)
    C = K // 256          # DoubleRow contraction chunks
    C2 = K // 512         # byte-quad chunks of the transposed weights
    KP = K // 2           # packed weight columns (bytes)
    TO = T // TN          # own token tiles (amax sharded among group of TN)

    nc = bacc.Bacc("TRN2", target_bir_lowering=False, debug=False,
                   num_devices=N_CORES)

    x_d = nc.dram_tensor("x", [ML, K], F16, kind="ExternalInput")  # reversed!
    xo_d = nc.dram_tensor("xown", [TO * 128, K], F16, kind="ExternalInput")
    # packed int4 bytes viewed as fp16 on the host (byte quads)
    wp_d = nc.dram_tensor("wp", [NS, KP // 2], F16, kind="ExternalInput")
    ws_d = nc.dram_tensor("wsc", [1, NS], F16, kind="ExternalInput")
    b_d = nc.dram_tensor("bias", [1, NS], F16, kind="ExternalInput")
    y_d = nc.dram_tensor("y", [ML, NS], F16, kind="ExternalOutput")
    if use_cc:
        cc_in = nc.dram_tensor("cc_in", [1, TO * 128], F32)
        cc_out = nc.dram_tensor("cc_out", [TN, TO * 128], F32)
        groups = [[g * TN + r for r in range(TN)] for g in range(TM)]
    if debug_dump:
        wsep_d = nc.dram_tensor("wsep_dump", [128, C * 2 * NS], I8,
                                kind="ExternalOutput")
        qt_d = nc.dram_tensor("qt_dump", [128, T * C * 256], I8,
                              kind="ExternalOutput")

    with TileContext(nc) as tc:
        with (
            tc.tile_pool(name="const", bufs=1) as cpool,
            tc.tile_pool(name="wsetup", bufs=2) as wpool,
            tc.tile_pool(name="xwork", bufs=3) as xpool,
            tc.tile_pool(name="qtp", bufs=qt_bufs) as qpool,
            tc.tile_pool(name="small", bufs=3) as spool,
            tc.tile_pool(name="epi", bufs=4) as epool,
            tc.tile_pool(name="psum", bufs=mm_bufs, space="PSUM") as ppool,
        ):
            # ---------------- constants ----------------
            wsc_row = cpool.tile([1, NS], F16)
            nc.sync.dma_start(wsc_row[:, :], ws_d.ap())
            wsc_bc = cpool.tile([128, NS], F16)
            nc.gpsimd.partition_broadcast(wsc_bc[:, :], wsc_row[:, :])
            bias_row = cpool.tile([1, NS], F16)
            nc.sync.dma_start(bias_row[:, :], b_d.ap())
            bias_bc = cpool.tile([128, NS], F16)
            nc.gpsimd.partition_broadcast(bias_bc[:, :], bias_row[:, :])
            # anti-diagonal J for the partition flip
            jm = cpool.tile([128, 128], F32)
            nc.vector.memset(jm[:, :], 1.0)
            nc.gpsimd.affine_select(jm[:, :], jm[:, :], pattern=[[1, 128]],
                                    base=-127, channel_multiplier=1,
                                    compare_op=AOP.is_equal, fill=0.0)

            # ---------------- x prefetch (overlap with setup) ----------
            pre_x = {}
            for i in range(n_pre):
                xt = xpool.tile([128, K], F16, tag="xpre", bufs=n_pre,
                                name=f"xt_0_{i}")
                nc.sync.dma_start(xt[:, :], x_d[i * 128:(i + 1) * 128, :])
                pre_x[i] = xt

            # ---------------- weight setup ----------------
            # one DRAM->SBUF transpose of fp16-viewed packed bytes:
            # wpT[jj, c2, n] holds bytes (k=512c2+4jj+{0,1}, k=...+{2,3})
            # of W[n, .]; then nibble-unpack into the two separated
            # k-planes per chunk that the DoubleRow moving operand wants.
            wpT = cpool.tile([128, C2, NS], F16)
            nc.sync.dma_start_transpose(wpT[:, :, :], wp_d.ap())
            wpT8 = wpT[:, :, :].bitcast(I8)  # [128, C2, 2*NS]
            wt_sep = cpool.tile([128, C, 2, NS], F8)
            for c2 in range(C2):
                bt = wpT8[:, c2, :].rearrange("p (n two) -> p two n", two=2)
                for h in range(2):
                    cp = 2 * c2 + h
                    src = bt[:, h, :]           # [128, NS] packed bytes
                    # low nibble: ((b & 15) ^ 8) - 8  -> plane e=0
                    lo4 = wpool.tile([128, NS], I8, tag="lo4")
                    nc.vector.tensor_scalar(lo4[:, :], src, 15, 8,
                                            op0=AOP.bitwise_and,
                                            op1=AOP.bitwise_xor)
                    nc.vector.tensor_scalar(wt_sep[:, cp, 0, :], lo4[:, :],
                                            8.0, None, op0=AOP.subtract)
                    # high nibble = floor(b/16) (sign-extended) -> plane e=1
                    hb = wpool.tile([128, NS], F16, tag="hb")
                    nc.scalar.activation(hb[:, :], src, ACTF.Copy,
                                         bias=1535.53125, scale=1.0 / 16)
                    nc.scalar.activation(wt_sep[:, cp, 1, :], hb[:, :],
                                         ACTF.Copy, bias=-1536.0, scale=1.0)

            if debug_dump:
                nc.sync.dma_start(
                    wsep_d.ap().rearrange("p (c two n) -> p c two n",
                                          c=C, two=2),
                    wt_sep[:, :, :, :].bitcast(I8))

            # ---------------- phase A: own-token amax ----------------
            s_own = spool.tile([128, TO], F32, tag="sown", bufs=1)
            for j in range(TO):
                xt = xpool.tile([128, K], F16, tag="x", bufs=x_bufs,
                                name=f"xta_{j}")
                nc.sync.dma_start(xt[:, :], xo_d[j * 128:(j + 1) * 128, :])
                xa = xpool.tile([128, K], I16, tag="xa", bufs=2,
                                name=f"xa_{j}")
                nc.vector.tensor_scalar(xa[:, :], xt[:, :].bitcast(I16),
                                        0x7FFF, None, op0=AOP.bitwise_and)
                w = K // 2
                while w >= 512:
                    nc.vector.tensor_tensor(xa[:, :w], xa[:, :w],
                                            xa[:, w:2 * w], op=AOP.max)
                    w //= 2
                mbits = spool.tile([128, 1], I16, tag="mbits")
                nc.vector.tensor_reduce(mbits[:, :], xa[:, :2 * w],
                                        axis=mybir.AxisListType.X,
                                        op=AOP.max)
                nc.vector.tensor_scalar(s_own[:, j:j + 1],
                                        mbits[:, :].bitcast(F16),
                                        1e-6, 1.0 / 7.0,
                                        op0=AOP.max, op1=AOP.mult)

            # share scales (in x_rev row order) across the token group
            s_rev = cpool.tile([128, T], F32)
            if use_cc:
                nc.sync.dma_start(
                    cc_in.ap().rearrange("o (j p) -> o p j", p=128),
                    s_own[:, :])
                nc.gpsimd.collective_compute(
                    "AllGather", AOP.bypass,
                    replica_groups=groups,
                    ins=[cc_in.ap()], outs=[cc_out.ap()])
                nc.sync.dma_start(
                    s_rev[:, :],
                    cc_out.ap().rearrange("r (j p) -> p (r j)", p=128))
            else:
                nc.vector.tensor_copy(s_rev[:, :TO], s_own[:, :])

            # flipped scales for the epilogue + reciprocal for quantization
            sq_all = cpool.tile([128, T], F32)
            nc.vector.reciprocal(sq_all[:, :], s_rev[:, :])
            ps_j = ppool.tile([128, T], F32, tag="psj", bufs=1)
            nc.tensor.matmul(ps_j[:, :], jm[:, :], s_rev[:, :],
                             start=True, stop=True)
            s_flip = cpool.tile([128, T], F32)
            nc.vector.tensor_copy(s_flip[:, :], ps_j[:, :])

            # ---------------- main loop ----------------
            if ablate == "pe":
                qT_const = cpool.tile([128, C, 128], F16)
                nc.vector.memset(qT_const[:, :, :], 0.0)
            for rep in range(repeat):
              for i in range(T):
                  if ablate != "pe":
                      if rep == 0 and i in pre_x:
                          xt = pre_x.pop(i)
                      else:
                          xt = xpool.tile([128, K], F16, tag="x", bufs=x_bufs,
                                          name=f"xt_{rep}_{i}")
                          nc.sync.dma_start(xt[:, :],
                                            x_d[i * 128:(i + 1) * 128, :])
                      # qb = fp16(x*sq + 1536): exact RNE integer round
                      nc.vector.tensor_scalar(xt[:, :], xt[:, :],
                                              sq_all[:, i:i + 1], 1536.0,
                                              op0=AOP.mult, op1=AOP.add)
                      # q8 = fp8(qb - 1536), alternating ACT/DVE.  The
                      # sigma k-swizzle that matches the weight quad
                      # layout is applied to x's columns on the HOST
                      # (amax is permutation-invariant, quant is
                      # elementwise), so this stays contiguous.
                      q8 = xpool.tile([128, K], F8, tag="q8",
                                      name=f"q8_{rep}_{i}")
                      if i % 3 != 2:
                          nc.scalar.activation(q8[:, :], xt[:, :], ACTF.Copy,
                                               bias=-1536.0, scale=1.0)
                      else:
                          nc.vector.tensor_scalar(q8[:, :], xt[:, :], 1536.0,
                                                  None, op0=AOP.subtract)
                      # pair-transpose: qT[jj, c, 2f+e] = q8[f, 256c+2jj+e]
                      qT = qpool.tile([128, C, 128], F16, tag="qT",
                                      name=f"qT_{rep}_{i}")
                      nc.scalar.dma_start_transpose(qT[:, :, :],
                                                    q8[:, :].bitcast(F16))
                      qT8 = qT[:, :, :].bitcast(F8)  # [128, C, 256]
                      if debug_dump and rep == 0:
                          nc.sync.dma_start(
                              qt_d[:, i * C * 256:(i + 1) * C * 256]
                              .rearrange("p (c n) -> p c n", c=C),
                              qT8[:, :, :].bitcast(I8))
                  else:
                      qT8 = qT_const[:, :, :].bitcast(F8)

                  for h in range(2):
                      if ablate == "nomm" and not (rep == 0 and i == 0):
                          sl = slice(h * NH, (h + 1) * NH)
                          ps = first_ps
                          t1 = epool.tile([128, NH], F16, tag="t1",
                                          name=f"t1_{rep}_{i}_{h}")
                          nc.vector.scalar_tensor_tensor(
                              t1[:, :], ps[:, :], s_flip[:, i:i + 1],
                              wsc_bc[:, sl], op0=AOP.mult, op1=AOP.mult)
                          yo = epool.tile([128, NH], F16, tag="yo",
                                          name=f"yo_{rep}_{i}_{h}")
                          nc.vector.tensor_tensor(yo[:, :], t1[:, :],
                                                  bias_bc[:, sl], op=AOP.add)
                          nc.gpsimd.dma_start(
                              y_d[ML - 128 * (i + 1):ML - 128 * i, sl],
                              yo[:, :])
                          continue
                      sl = slice(h * NH, (h + 1) * NH)
                      ps = ppool.tile([128, NH], F32, tag="mm", bufs=mm_bufs,
                                      name=f"ps_{rep}_{i}_{h}")
                      if rep == 0 and i == 0 and h == 0:
                          first_ps = ps
                      for c in range(C):
                          nc.tensor.matmul(ps[:, :], qT8[:, c, :],
                                           wt_sep[:, c, :, sl],
                                           start=(c == 0), stop=(c == C - 1),
                                           perf_mode=SWI)
                      # epilogue: y = (ps * a_scale) * wscale + bias
                      t1 = epool.tile([128, NH], F16, tag="t1",
                                      name=f"t1_{rep}_{i}_{h}")
                      nc.vector.scalar_tensor_tensor(
                          t1[:, :], ps[:, :], s_flip[:, i:i + 1],
                          wsc_bc[:, sl], op0=AOP.mult, op1=AOP.mult)
                      yo = epool.tile([128, NH], F16, tag="yo",
                                      name=f"yo_{rep}_{i}_{h}")
                      nc.vector.tensor_tensor(yo[:, :], t1[:, :],
                                              bias_bc[:, sl], op=AOP.add)
                      nc.gpsimd.dma_start(
                          y_d[ML - 128 * (i + 1):ML - 128 * i, sl], yo[:, :])

    nc.compile()
    return nc


_CACHE = {}


def _get_nc():
    if "nc" not in _CACHE:
        _CACHE["nc"] = build()
    return _CACHE["nc"]


_PERM = {}


def _sigma_inv(K):
    """Column order for x such that the contiguous q8 byte stream matches
    the weight quad layout: byte position 512B+256h+2jj+e must hold
    k = 512B+4jj+2h+e."""
    if K not in _PERM:
        pos = np.arange(K)
        B = pos // 512
        t = pos % 512
        h = t // 256
        jj = (t % 256) // 2
        e = t % 2
        _PERM[K] = 512 * B + 4 * jj + 2 * h + e
    return _PERM[K]


def _in_maps(x, qweight_packed, w_scales, bias):
    M, K, N = 4096, 4096, 4096
    ML = M // TM
    NS = N // TN
    MO = ML // TN
    x2 = np.asarray(x).reshape(M, K)[:, _sigma_inv(K)]
    x_rev = np.ascontiguousarray(x2[::-1])
    wsc = np.asarray(w_scales).reshape(N)
    bias = np.asarray(bias).reshape(N)
    in_maps = []
    for c in range(N_CORES):
        g, r = divmod(c, TN)
        xg = x_rev[g * ML:(g + 1) * ML]
        sl = slice(r * NS, (r + 1) * NS)
        wp8 = np.ascontiguousarray(np.asarray(qweight_packed)[sl])
        in_maps.append({
            "x": xg,
            "xown": np.ascontiguousarray(xg[r * MO:(r + 1) * MO]),
            "wp": wp8.view(np.float16),
            "wsc": np.ascontiguousarray(wsc[sl]).reshape(1, NS),
            "bias": np.ascontiguousarray(bias[sl]).reshape(1, NS),
        })
    return in_maps


def kernel(x, qweight_packed, w_scales, bias):
    N = 4096
    nc = _get_nc()
    in_maps = _in_maps(x, qweight_packed, w_scales, bias)
    res = run_bass_kernel_spmd(nc, in_maps, core_ids=list(range(N_CORES)))
    # group g=0 (cores 0-3) produced original tokens [2048:4096),
    # group g=1 (cores 4-7) produced original tokens [0:2048).
    top = np.concatenate([res.results[TN + r]["y"] for r in range(TN)],
                         axis=1)
    bot = np.concatenate([res.results[r]["y"] for r in range(TN)], axis=1)
    y = np.concatenate([top, bot], axis=0)
    return y.reshape(2, 2048, N)


# revision 3
# speedup vs baseline: 1.0226x; 1.0226x over previous
"""W4A4 quantized linear (AutoQVLALinearW4A4) on 8 Trainium2 NeuronCores.

y = dequant_rowwise_quant(x) @ dequant_w4(qweight)^T + bias

2x4 sharding: 2 token groups x 4 out-feature groups. Each core gets a
[2048, 4096] slice of x (row-reversed) and a 1024-row slice of the packed
weights / scales / bias, and produces a [2048, 1024] slice of the output.
vs pure column-parallel this halves both the per-core HBM x traffic and
the SBUF-fabric transpose traffic, bringing the DMA pipeline (~85us) well
under the fp8 PE roofline (~110-125us).

Device algorithm (per core), exact-integer math on the PE:
  1. weights: ONE DMA-transpose of the packed int4 buffer straight from
     DRAM (viewed as fp16 so byte QUADS move as units): partition jj of
     quad-chunk c2 holds bytes for k = 512*c2 + 4*jj + {0,1,2,3} of all
     1024 local out-features. Nibble-unpack on ACT (high) / DVE (low)
     directly into wt_sep[jj, c', e, n], c' = 2*c2+h, e = nibble plane.
  2. amax for OWN 512 tokens (int16 abs-bit trick + max tree on DVE),
     a_scale = max/7, AllGather of the 4 group members' a_scales (2KB).
  3. qb = fp16(x * (1/a_scale) + 1536)  (exact round-half-even to int+1536)
     q8 = fp8_e4m3(qb - 1536), written with the k-swizzle sigma that makes
     the q8 byte order match the weight quad layout: byte
     512*B + 256*h + 2*jj + e  <-  k = 512*B + 4*jj + 2*h + e.
     (A pure permutation of the contraction; the matmul sum is invariant.)
  4. transpose q8 via DMA-transpose of byte-PAIRS viewed as fp16, then
     fp8 DoubleRowSwInterleave matmuls: q^T byte-pairs are the STATIONARY
     operand (SwInterleave's column reversal is cancelled by feeding x
     row-reversed from the host), wt_sep planes are the MOVING operand.
     Two PSUM banks per token tile cover the 1024 local out-features.
  5. epilogue: (psum * a_scale_pp) * wscale_bcast + bias_bcast on DVE.
     a_scale needs a partition flip (out tokens run opposite to the
     x_rev rows) -> one tiny anti-diagonal matmul (J @ s).
"""

import numpy as np
import concourse.bass as bass
import concourse.mybir as mybir
from concourse import bacc
from concourse.tile import TileContext
from concourse.bass_utils import run_bass_kernel_spmd

F8 = mybir.dt.float8e4
F16 = mybir.dt.float16
F32 = mybir.dt.float32
I8 = mybir.dt.int8
I16 = mybir.dt.int16
AOP = mybir.AluOpType
ACTF = mybir.ActivationFunctionType
SWI = mybir.MatmulPerfMode.DoubleRowSwInterleave

N_CORES = 8
TM = 2            # token groups
TN = 4            # out-feature groups


def build(ML=2048, K=4096, NS=1024, use_cc=True, mm_bufs=6, qt_bufs=8,
          x_bufs=6, n_pre=3, repeat=1, debug_dump=False, ablate=None):
    """Build + compile the per-core program. Returns the Bacc object."""
    assert ML % 128 == 0 and K % 512 == 0 and NS % 256 == 0
    T = ML // 128         # token tiles
    NH = NS // 2          # out-feature half (one PSUM bank)
    C = K // 256          # DoubleRow contraction chunks
    C2 = K // 512         # byte-quad chunks of the transposed weights
    KP = K // 2           # packed weight columns (bytes)
    TO = T // TN          # own token tiles (amax sharded among group of TN)

    nc = bacc.Bacc("TRN2", target_bir_lowering=False, debug=False,
                   num_devices=N_CORES)

    x_d = nc.dram_tensor("x", [ML, K], F16, kind="ExternalInput")  # reversed!
    xo_d = nc.dram_tensor("xown", [TO * 128, K], F16, kind="ExternalInput")
    # packed int4 bytes viewed as fp16 on the host (byte quads)
    wp_d = nc.dram_tensor("wp", [NS, KP // 2], F16, kind="ExternalInput")
    ws_d = nc.dram_tensor("wsc", [1, NS], F16, kind="ExternalInput")
    b_d = nc.dram_tensor("bias", [1, NS], F16, kind="ExternalInput")
    y_d = nc.dram_tensor("y", [ML, NS], F16, kind="ExternalOutput")
    if use_cc:
        cc_in = nc.dram_tensor("cc_in", [1, TO * 128], F32)
        cc_out = nc.dram_tensor("cc_out", [TN, TO * 128], F32)
        groups = [[g * TN + r for r in range(TN)] for g in range(TM)]
    if debug_dump:
        wsep_d = nc.dram_tensor("wsep_dump", [128, C * 2 * NS], I8,
                                kind="ExternalOutput")
        qt_d = nc.dram_tensor("qt_dump", [128, T * C * 256], I8,
                              kind="ExternalOutput")

    with TileContext(nc) as tc:
        with (
            tc.tile_pool(name="const", bufs=1) as cpool,
            tc.tile_pool(name="wsetup", bufs=2) as wpool,
            tc.tile_pool(name="xwork", bufs=3) as xpool,
            tc.tile_pool(name="qtp", bufs=qt_bufs) as qpool,
            tc.tile_pool(name="small", bufs=3) as spool,
            tc.tile_pool(name="epi", bufs=4) as epool,
            tc.tile_pool(name="psum", bufs=mm_bufs, space="PSUM") as ppool,
        ):
            # ---------------- constants ----------------
            wsc_row = cpool.tile([1, NS], F16)
            nc.sync.dma_start(wsc_row[:, :], ws_d.ap())
            wsc_bc = cpool.tile([128, NS], F16)
            nc.gpsimd.partition_broadcast(wsc_bc[:, :], wsc_row[:, :])
            bias_row = cpool.tile([1, NS], F16)
            nc.sync.dma_start(bias_row[:, :], b_d.ap())
            bias_bc = cpool.tile([128, NS], F16)
            nc.gpsimd.partition_broadcast(bias_bc[:, :], bias_row[:, :])
            # anti-diagonal J for the partition flip
            jm = cpool.tile([128, 128], F32)
            nc.vector.memset(jm[:, :], 1.0)
            nc.gpsimd.affine_select(jm[:, :], jm[:, :], pattern=[[1, 128]],
                                    base=-127, channel_multiplier=1,
                                    compare_op=AOP.is_equal, fill=0.0)

            # ---------------- x prefetch (overlap with setup) ----------
            pre_x = {}
            for i in range(n_pre):
                xt = xpool.tile([128, K], F16, tag="xpre", bufs=n_pre,
                                name=f"xt_0_{i}")
                nc.sync.dma_start(xt[:, :], x_d[i * 128:(i + 1) * 128, :])
                pre_x[i] = xt

            # ---------------- weight setup ----------------
            # one DRAM->SBUF transpose of fp16-viewed packed bytes:
            # wpT[jj, c2, n] holds bytes (k=512c2+4jj+{0,1}, k=...+{2,3})
            # of W[n, .]; then nibble-unpack into the two separated
            # k-planes per chunk that the DoubleRow moving operand wants.
            wpT = cpool.tile([128, C2, NS], F16)
            nc.sync.dma_start_transpose(wpT[:, :, :], wp_d.ap())
            wpT8 = wpT[:, :, :].bitcast(I8)  # [128, C2, 2*NS]
            wt_sep = cpool.tile([128, C, 2, NS], F8)
            for c2 in range(C2):
                bt = wpT8[:, c2, :].rearrange("p (n two) -> p two n", two=2)
                for h in range(2):
                    cp = 2 * c2 + h
                    src = bt[:, h, :]           # [128, NS] packed bytes
                    # low nibble: ((b & 15) ^ 8) - 8  -> plane e=0
                    lo4 = wpool.tile([128, NS], I8, tag="lo4")
                    nc.vector.tensor_scalar(lo4[:, :], src, 15, 8,
                                            op0=AOP.bitwise_and,
                                            op1=AOP.bitwise_xor)
                    nc.vector.tensor_scalar(wt_sep[:, cp, 0, :], lo4[:, :],
                                            8.0, None, op0=AOP.subtract)
                    # high nibble = floor(b/16) (sign-extended) -> plane e=1
                    hb = wpool.tile([128, NS], F16, tag="hb")
                    nc.scalar.activation(hb[:, :], src, ACTF.Copy,
                                         bias=1535.53125, scale=1.0 / 16)
                    nc.scalar.activation(wt_sep[:, cp, 1, :], hb[:, :],
                                         ACTF.Copy, bias=-1536.0, scale=1.0)

            if debug_dump:
                nc.sync.dma_start(
                    wsep_d.ap().rearrange("p (c two n) -> p c two n",
                                          c=C, two=2),
                    wt_sep[:, :, :, :].bitcast(I8))

            # ---------------- phase A: own-token amax ----------------
            s_own = spool.tile([128, TO], F32, tag="sown", bufs=1)
            for j in range(TO):
                xt = xpool.tile([128, K], F16, tag="x", bufs=x_bufs,
                                name=f"xta_{j}")
                nc.sync.dma_start(xt[:, :], xo_d[j * 128:(j + 1) * 128, :])
                xa = xpool.tile([128, K], I16, tag="xa", bufs=2,
                                name=f"xa_{j}")
                nc.vector.tensor_scalar(xa[:, :], xt[:, :].bitcast(I16),
                                        0x7FFF, None, op0=AOP.bitwise_and)
                w = K // 2
                while w >= 512:
                    nc.vector.tensor_tensor(xa[:, :w], xa[:, :w],
                                            xa[:, w:2 * w], op=AOP.max)
                    w //= 2
                mbits = spool.tile([128, 1], I16, tag="mbits")
                nc.vector.tensor_reduce(mbits[:, :], xa[:, :2 * w],
                                        axis=mybir.AxisListType.X,
                                        op=AOP.max)
                nc.vector.tensor_scalar(s_own[:, j:j + 1],
                                        mbits[:, :].bitcast(F16),
                                        1e-6, 1.0 / 7.0,
                                        op0=AOP.max, op1=AOP.mult)

            # share scales (in x_rev row order) across the token group
            s_rev = cpool.tile([128, T], F32)
            if use_cc:
                nc.sync.dma_start(
                    cc_in.ap().rearrange("o (j p) -> o p j", p=128),
                    s_own[:, :])
                nc.gpsimd.collective_compute(
                    "AllGather", AOP.bypass,
                    replica_groups=groups,
                    ins=[cc_in.ap()], outs=[cc_out.ap()])
                nc.sync.dma_start(
                    s_rev[:, :],
                    cc_out.ap().rearrange("r (j p) -> p (r j)", p=128))
            else:
                nc.vector.tensor_copy(s_rev[:, :TO], s_own[:, :])

            # flipped scales for the epilogue + reciprocal for quantization
            sq_all = cpool.tile([128, T], F32)
            nc.vector.reciprocal(sq_all[:, :], s_rev[:, :])
            ps_j = ppool.tile([128, T], F32, tag="psj", bufs=1)
            nc.tensor.matmul(ps_j[:, :], jm[:, :], s_rev[:, :],
                             start=True, stop=True)
            s_flip = cpool.tile([128, T], F32)
            nc.vector.tensor_copy(s_flip[:, :], ps_j[:, :])

            # ---------------- main loop ----------------
            if ablate == "pe":
                qT_const = cpool.tile([128, C, 128], F16)
                nc.vector.memset(qT_const[:, :, :], 0.0)
            for rep in range(repeat):
              for i in range(T):
                  if ablate != "pe":
                      if rep == 0 and i in pre_x:
                          xt = pre_x.pop(i)
                      else:
                          xt = xpool.tile([128, K], F16, tag="x", bufs=x_bufs,
                                          name=f"xt_{rep}_{i}")
                          nc.sync.dma_start(xt[:, :],
                                            x_d[i * 128:(i + 1) * 128, :])
                      # qb = fp16(x*sq + 1536): exact RNE integer round
                      nc.vector.tensor_scalar(xt[:, :], xt[:, :],
                                              sq_all[:, i:i + 1], 1536.0,
                                              op0=AOP.mult, op1=AOP.add)
                      # q8 = fp8(qb - 1536), alternating ACT/DVE.  The
                      # sigma k-swizzle that matches the weight quad
                      # layout is applied to x's columns on the HOST
                      # (amax is permutation-invariant, quant is
                      # elementwise), so this stays contiguous.
                      q8 = xpool.tile([128, K], F8, tag="q8",
                                      name=f"q8_{rep}_{i}")
                      if i % 2 != 1:
                          nc.scalar.activation(q8[:, :], xt[:, :], ACTF.Copy,
                                               bias=-1536.0, scale=1.0)
                      else:
                          nc.vector.tensor_scalar(q8[:, :], xt[:, :], 1536.0,
                                                  None, op0=AOP.subtract)
                      # pair-transpose: qT[jj, c, 2f+e] = q8[f, 256c+2jj+e]
                      qT = qpool.tile([128, C, 128], F16, tag="qT",
                                      name=f"qT_{rep}_{i}")
                      nc.scalar.dma_start_transpose(qT[:, :, :],
                                                    q8[:, :].bitcast(F16))
                      qT8 = qT[:, :, :].bitcast(F8)  # [128, C, 256]
                      if debug_dump and rep == 0:
                          nc.sync.dma_start(
                              qt_d[:, i * C * 256:(i + 1) * C * 256]
                              .rearrange("p (c n) -> p c n", c=C),
                              qT8[:, :, :].bitcast(I8))
                  else:
                      qT8 = qT_const[:, :, :].bitcast(F8)

                  for h in range(2):
                      if ablate == "nomm" and not (rep == 0 and i == 0):
                          sl = slice(h * NH, (h + 1) * NH)
                          ps = first_ps
                          t1 = epool.tile([128, NH], F16, tag="t1",
                                          name=f"t1_{rep}_{i}_{h}")
                          nc.vector.scalar_tensor_tensor(
                              t1[:, :], ps[:, :], s_flip[:, i:i + 1],
                              wsc_bc[:, sl], op0=AOP.mult, op1=AOP.mult)
                          yo = epool.tile([128, NH], F16, tag="yo",
                                          name=f"yo_{rep}_{i}_{h}")
                          nc.vector.tensor_tensor(yo[:, :], t1[:, :],
                                                  bias_bc[:, sl], op=AOP.add)
                          nc.gpsimd.dma_start(
                              y_d[ML - 128 * (i + 1):ML - 128 * i, sl],
                              yo[:, :])
                          continue
                      sl = slice(h * NH, (h + 1) * NH)
                      ps = ppool.tile([128, NH], F32, tag="mm", bufs=mm_bufs,
                                      name=f"ps_{rep}_{i}_{h}")
                      if rep == 0 and i == 0 and h == 0:
                          first_ps = ps
                      for c in range(C):
                          nc.tensor.matmul(ps[:, :], qT8[:, c, :],
                                           wt_sep[:, c, :, sl],
                                           start=(c == 0), stop=(c == C - 1),
                                           perf_mode=SWI)
                      # epilogue: y = (ps * a_scale) * wscale + bias
                      t1 = epool.tile([128, NH], F16, tag="t1",
                                      name=f"t1_{rep}_{i}_{h}")
                      nc.vector.scalar_tensor_tensor(
                          t1[:, :], ps[:, :], s_flip[:, i:i + 1],
                          wsc_bc[:, sl], op0=AOP.mult, op1=AOP.mult)
                      yo = epool.tile([128, NH], F16, tag="yo",
                                      name=f"yo_{rep}_{i}_{h}")
                      nc.vector.tensor_tensor(yo[:, :], t1[:, :],
                                              bias_bc[:, sl], op=AOP.add)
                      nc.gpsimd.dma_start(
                          y_d[ML - 128 * (i + 1):ML - 128 * i, sl], yo[:, :])

    nc.compile()
    return nc


_CACHE = {}


def _get_nc():
    if "nc" not in _CACHE:
        _CACHE["nc"] = build()
    return _CACHE["nc"]


_PERM = {}


def _sigma_inv(K):
    """Column order for x such that the contiguous q8 byte stream matches
    the weight quad layout: byte position 512B+256h+2jj+e must hold
    k = 512B+4jj+2h+e."""
    if K not in _PERM:
        pos = np.arange(K)
        B = pos // 512
        t = pos % 512
        h = t // 256
        jj = (t % 256) // 2
        e = t % 2
        _PERM[K] = 512 * B + 4 * jj + 2 * h + e
    return _PERM[K]


def _in_maps(x, qweight_packed, w_scales, bias):
    M, K, N = 4096, 4096, 4096
    ML = M // TM
    NS = N // TN
    MO = ML // TN
    x2 = np.asarray(x).reshape(M, K)[:, _sigma_inv(K)]
    x_rev = np.ascontiguousarray(x2[::-1])
    wsc = np.asarray(w_scales).reshape(N)
    bias = np.asarray(bias).reshape(N)
    in_maps = []
    for c in range(N_CORES):
        g, r = divmod(c, TN)
        xg = x_rev[g * ML:(g + 1) * ML]
        sl = slice(r * NS, (r + 1) * NS)
        wp8 = np.ascontiguousarray(np.asarray(qweight_packed)[sl])
        in_maps.append({
            "x": xg,
            "xown": np.ascontiguousarray(xg[r * MO:(r + 1) * MO]),
            "wp": wp8.view(np.float16),
            "wsc": np.ascontiguousarray(wsc[sl]).reshape(1, NS),
            "bias": np.ascontiguousarray(bias[sl]).reshape(1, NS),
        })
    return in_maps


def kernel(x, qweight_packed, w_scales, bias):
    N = 4096
    nc = _get_nc()
    in_maps = _in_maps(x, qweight_packed, w_scales, bias)
    res = run_bass_kernel_spmd(nc, in_maps, core_ids=list(range(N_CORES)))
    # group g=0 (cores 0-3) produced original tokens [2048:4096),
    # group g=1 (cores 4-7) produced original tokens [0:2048).
    top = np.concatenate([res.results[TN + r]["y"] for r in range(TN)],
                         axis=1)
    bot = np.concatenate([res.results[r]["y"] for r in range(TN)], axis=1)
    y = np.concatenate([top, bot], axis=0)
    return y.reshape(2, 2048, N)
